# revision 22
# baseline (speedup 1.0000x reference)
"""MDCA loss kernel for Trainium2 (8 NeuronCores, SPMD data-parallel).

Problem: 4 CAMs [128, 1000, 14, 14] f32 + target [128] i64 ->
4 scalar losses: mean_c |mean_{b,h,w} cam[b,c,h,w] - bincount(target)[c]/B|.

Strategy (memory-bound, ~401 MB total input):
  - Shard batch across 8 cores: 16 rows/core, ~50 MB/core.
  - Per core, per cam: view the [16, 196000] shard as [125p, 16b, 1568]
    where partition p holds classes 8p..8p+7 (1568 = 8 classes * 196 hw,
    contiguous in DRAM -> 6.3 KB DMA runs). One batch row per DMA load;
    DVE reduce_sum each tile [125, 8, 196] -> [125, 8] per-class partials,
    then a tiny second reduce over the 16 batch rows -> per-class sums.
  - Loads are striped 8:8 across TWO DMA queues -- the SP (sync) HWDGE
    ring (f32) and the gpsimd SWDGE ring, which CASTS its rows to bf16
    in the DMA. Together the rings sustain ~850 GB/s/core (one ring
    alone: ~400; the Activation HWDGE ring is ~3.7x slower for bulk
    loads and carries only the tiny output stores).
  - The otherwise-idle Activation ENGINE re-casts each f32 tile into a
    rotating bf16 scratch tile (one Copy op each) so DVE reduces every
    tile in 16-bit 2x mode (~49 us/iter instead of the ~94 us f32
    full-touch that was the critical path); the sync ring (~63 us/iter)
    now binds. bf16-quantizing the inputs perturbs this loss by only
    ~4e-5 relative -- far inside the 2e-2 gate.
  - Stage1 uses a 3-D access pattern ([p, cc, xx], contiguous stage
    writes); the 4-D batched form costs ~2x on hardware.
  - One [125, 8] f32 output DMA per cam per core; host sums the 8 core
    partials, adds bincount(target), and computes the 4 scalar losses.

Raw Bass Block (not Tile): HWDGE DMA instructions only support one inline
sync-wait, so semaphores are placed by hand -- one completion sem per SBUF
slot (concurrent DMAs always target distinct slots), WAR on slot reuse
guarded transitively through the DVE sem.
"""

import numpy as np

B, C, H, W = 128, 1000, 14, 14
HWSZ = H * W                 # 196
N_CORES = 8
B_SH = B // N_CORES          # 16 batch rows per core
P = 125                      # partitions used; class c -> (p=c//8, cc=c%8)
CC = 8                       # classes per partition
RUN = CC * HWSZ              # 1568 contiguous f32 per (p, b)
F = C * HWSZ                 # 196000 elements per batch row
N_CAMS = 4

N_BUFS = 16                  # SBUF slots (6272 B/partition each)
LOAD_RINGS = ("sync", "gpsimd")
# ring assignment per (load index % 16): 0 -> sync (f32), 1 -> gpsimd
# (casts to bf16). 7:9 split — the bf16 ring takes slightly more than half
# because its tiles cost DVE half as much, balancing DVE against both DMA
# rings (measured: sync ~400 GB/s, gpsimd ~430 GB/s, DVE ~1 f32 or
# 2 bf16 elems/lane/cycle).
STRIPE = (0, 1, 0, 1, 1, 0, 1, 0, 1, 0, 1, 1, 0, 1, 0, 1)
# act_cast mode uses an even split instead: with the Act engine re-casting
# the sync ring's f32 tiles to bf16 (one Copy op per tile into a rotating
# scratch pool), DVE reduces every tile in 16-bit 2x mode and stops being
# the critical path; the rings rebalance to 8:8.
STRIPE_EVEN = (0, 1) * 8
N_SCRATCH = 8                # rotating bf16 scratch tiles for act_cast

_CACHE = {}


def _build_nc(n_bufs=None, n_iters=1, load_rings=None, cast_ring=True,
              act_cast=True):
    from contextlib import ExitStack

    import concourse.bass as bass
    import concourse.mybir as mybir

    nb = N_BUFS if n_bufs is None else n_bufs
    rings = LOAD_RINGS if load_rings is None else load_rings
    # cast_ring: the gpsimd (SWDGE) ring casts its rows to bf16 in the DMA;
    # DVE reduces those tiles in 16-bit mode (2x) into f32 stages. Ring
    # assignment follows STRIPE (period 16); nb must be 16 so that a slot
    # is always refilled by the same ring (same dtype).
    assert not cast_ring or (
        len(rings) == 2 and rings[1] == "gpsimd" and nb == 16
    )
    n_chunks = B_SH                  # loads per cam (one batch row each)
    n_loads = N_CAMS * n_chunks      # loads per iteration
    dve_per_iter = n_loads + N_CAMS  # stage1 + stage2 ops per iteration

    def dve_after_s1(k):
        # dve_sem value right after stage1-reduce #k retires (DVE order per
        # cam: n_chunks * s1 then one s2)
        return k + k // n_chunks + 1

    def dve_after_s2(i):
        # dve_sem delta within an iteration once cam i's stage2 retired
        return (i + 1) * (n_chunks + 1)

    f32 = mybir.dt.float32
    bf16 = mybir.dt.bfloat16
    nc = bass.Bass()
    cams = [
        nc.dram_tensor(f"cam_{i}", [B_SH, F], f32, kind="ExternalInput")
        for i in range(N_CAMS)
    ]
    out = nc.dram_tensor("sums", [P, N_CAMS * CC], f32, kind="ExternalOutput")

    stripe = STRIPE_EVEN if act_cast else STRIPE

    def ring_of(n):
        if cast_ring:
            return stripe[n % 16]
        return n % len(rings)

    def slot_dtype(s):
        # slot s is always filled by ring stripe[s % 16] (n_loads % 16 == 0
        # and nb == 16, so load n -> slot n % 16 preserves the stripe)
        return bf16 if cast_ring and stripe[s % 16] == 1 else f32

    def is_f32_tile(c):
        return cast_ring and stripe[c % 16] == 0

    def act_pos(c):
        # rank of f32 tile c within its cam's f32 tiles (act-cast order)
        return sum(1 for c2 in range(c) if stripe[c2 % 16] == 0)

    n_f32 = sum(1 for c in range(16) if stripe[c] == 0)  # f32 tiles per cam

    with ExitStack() as ctx:
        bufs = [
            ctx.enter_context(nc.sbuf_tensor(f"t{s}", [P, RUN], slot_dtype(s)))
            for s in range(nb)
        ]
        stages = [
            ctx.enter_context(nc.sbuf_tensor(f"stage{i}", [P, n_chunks, CC], f32))
            for i in range(N_CAMS)
        ]
        out_sums = ctx.enter_context(nc.sbuf_tensor("osum", [P, N_CAMS * CC], f32))
        scratch = [
            ctx.enter_context(nc.sbuf_tensor(f"sc{s}", [P, RUN], bf16))
            for s in range(N_SCRATCH if act_cast else 0)
        ]
        # one completion sem per buffer slot: concurrent loads target distinct
        # slots, so "slot_sem >= 16*k" unambiguously means "k-th load into this
        # slot is fully complete" (each DMA is 16 sub-completions)
        slot_sems = [
            ctx.enter_context(nc.semaphore(f"slot_sem{s}")) for s in range(nb)
        ]
        out_sem = ctx.enter_context(nc.semaphore("out_sem"))
        dve_sem = ctx.enter_context(nc.semaphore("dve_sem"))
        act_sem = ctx.enter_context(nc.semaphore("act_sem"))
        block = ctx.enter_context(nc.Block())

        act_per_iter = N_CAMS * n_f32  # Act cast ops per iteration

        def act_count_after(g, i, c):
            # act_sem value once the cast of tile (iter g, cam i, chunk c)
            # has retired (Act processes f32 tiles in load order)
            return g * act_per_iter + i * n_f32 + act_pos(c) + 1

        def loader(eng, g, parity, nrings):
            # emit this engine's share of iteration g's loads (striped);
            # slot-reuse WAR is guarded transitively through whichever
            # engine consumes the slot (DVE reduce, or Act cast when
            # act_cast handles this f32 slot)
            for n in range(n_loads):
                if ring_of(n) != parity:
                    continue
                i, c = divmod(n, n_chunks)
                gn = g * n_loads + n
                if gn >= nb:
                    pk = gn - nb
                    pg, pn = divmod(pk, n_loads)
                    pi, pc = divmod(pn, n_chunks)
                    if act_cast and is_f32_tile(pc):
                        eng.wait_ge(act_sem, act_count_after(pg, pi, pc))
                    else:
                        eng.wait_ge(
                            dve_sem, pg * dve_per_iter + dve_after_s1(pn)
                        )
                src = cams[i][c:c + 1, :].rearrange(
                    "b (p x) -> p (b x)", p=P, x=RUN
                )
                eng.dma_start(bufs[gn % nb][:], src).then_inc(
                    slot_sems[gn % nb], 16
                )

        for r, ring in enumerate(rings):

            def ring_body(eng, r=r):
                for g in range(n_iters):
                    loader(eng, g, r, len(rings))

            getattr(block, ring)(ring_body)

        @block.scalar
        def _(scalar):
            for g in range(n_iters):
                if act_cast:
                    # re-cast each f32 tile to bf16 scratch so DVE reduces
                    # it in 16-bit 2x mode; one Copy op per tile
                    t = 0
                    for i in range(N_CAMS):
                        for c in range(n_chunks):
                            if not is_f32_tile(c):
                                continue
                            gt = g * act_per_iter + t
                            if gt >= N_SCRATCH:
                                # WAR: scratch slot's previous tile was
                                # stage1-reduced by DVE
                                pt = gt - N_SCRATCH
                                pg, pr = divmod(pt, act_per_iter)
                                pi, pp = divmod(pr, n_f32)
                                pc = [c2 for c2 in range(n_chunks)
                                      if is_f32_tile(c2)][pp]
                                scalar.wait_ge(
                                    dve_sem,
                                    pg * dve_per_iter
                                    + dve_after_s1(pi * n_chunks + pc),
                                )
                            gn = g * n_loads + i * n_chunks + c
                            scalar.wait_ge(
                                slot_sems[gn % nb], 16 * (gn // nb + 1)
                            )
                            nc.scalar.copy(
                                out=scratch[gt % N_SCRATCH][:],
                                in_=bufs[gn % nb][:],
                            ).then_inc(act_sem, 1)
                            t += 1
                # per-cam output stores, last iteration only (out_sums is
                # rewritten every iteration; only the final value is read,
                # and per-iter stores would barrier this queue between
                # iterations in act_cast mode)
                if g == n_iters - 1:
                    for i in range(N_CAMS):
                        scalar.wait_ge(
                            dve_sem, g * dve_per_iter + dve_after_s2(i)
                        )
                        scalar.dma_start(
                            out[:, i * CC:(i + 1) * CC],
                            out_sums[:, i * CC:(i + 1) * CC],
                        ).then_inc(out_sem, 16)
            scalar.wait_ge(out_sem, 16 * N_CAMS)

        @block.vector
        def _(vector):
            for g in range(n_iters):
                dve_base = g * dve_per_iter
                for i in range(N_CAMS):
                    for c in range(n_chunks):
                        n = i * n_chunks + c
                        gn = g * n_loads + n
                        if g > 0 and c == 0:
                            # WAR: stages[i] reread by prev iter's stage2
                            vector.wait_ge(
                                dve_sem,
                                (g - 1) * dve_per_iter + dve_after_s2(i),
                            )
                        if act_cast and is_f32_tile(c):
                            # tile was re-cast to bf16 scratch by Act
                            gt = g * act_per_iter + i * n_f32 + act_pos(c)
                            vector.wait_ge(
                                act_sem, act_count_after(g, i, c)
                            )
                            src_tile = scratch[gt % N_SCRATCH]
                        else:
                            vector.wait_ge(
                                slot_sems[gn % nb], 16 * (gn // nb + 1)
                            )
                            src_tile = bufs[gn % nb]
                        nc.vector.reduce_sum(
                            out=stages[i][:, c],
                            in_=src_tile[:].rearrange(
                                "p (cc xx) -> p cc xx", cc=CC
                            ),
                            axis=mybir.AxisListType.X,
                        ).then_inc(dve_sem, 1)
                    # reduce the 16 batch partials per class:
                    # [P, cc, h] -> [P, cc]; same-engine wait makes sure the
                    # stage1 writes retired before this read
                    vector.wait_ge(dve_sem, dve_base + (i + 1) * n_chunks + i)
                    nc.vector.reduce_sum(
                        out=out_sums[:, i * CC:(i + 1) * CC],
                        in_=stages[i][:].rearrange("p h cc -> p cc h"),
                        axis=mybir.AxisListType.X,
                    ).then_inc(dve_sem, 1)

    return nc


def _get_nc():
    if "nc" not in _CACHE:
        _CACHE["nc"] = _build_nc()
    return _CACHE["nc"]


def _run_on_device(in_maps, nc=None, **kwargs):
    from concourse.bass_utils import run_bass_kernel_spmd

    return run_bass_kernel_spmd(
        nc if nc is not None else _get_nc(),
        in_maps,
        core_ids=list(range(N_CORES)),
        **kwargs,
    )


def _make_in_maps(cams):
    in_maps = []
    for k in range(N_CORES):
        m = {}
        for i, cam in enumerate(cams):
            m[f"cam_{i}"] = np.ascontiguousarray(
                np.asarray(cam)[k * B_SH:(k + 1) * B_SH].reshape(B_SH, F),
                dtype=np.float32,
            )
        in_maps.append(m)
    return in_maps


def kernel(cam_0, cam_1, cam_2, cam_3, target, _bench_results=None, **_kw):
    in_maps = _make_in_maps((cam_0, cam_1, cam_2, cam_3))
    res = _run_on_device(in_maps)
    if _bench_results is not None:
        _bench_results.append(res)

    # host combine: [125, 32] per core -> per-class totals -> scalar losses
    counts = np.bincount(np.asarray(target).astype(np.int64), minlength=C)
    avg_count = counts.astype(np.float64) / B
    total = np.zeros((P, N_CAMS * CC), dtype=np.float64)
    for r in res.results:
        total += r["sums"].astype(np.float64)

    losses = []
    for i in range(N_CAMS):
        per_class = total[:, i * CC:(i + 1) * CC].reshape(C)  # index = 8p+cc = c
        avg_conf = per_class / (B * HWSZ)
        losses.append(np.float32(np.abs(avg_conf - avg_count).mean()))
    return tuple(np.asarray(l, dtype=np.float32) for l in losses)


# revision 24
# speedup vs baseline: 1.6746x; 1.6746x over previous
"""MDCA loss kernel for Trainium2 (8 NeuronCores, SPMD data-parallel).

Problem: 4 CAMs [128, 1000, 14, 14] f32 + target [128] i64 ->
4 scalar losses: mean_c |mean_{b,h,w} cam[b,c,h,w] - bincount(target)[c]/B|.

Strategy (memory-bound, ~401 MB total input):
  - Shard batch across 8 cores: 16 rows/core, ~50 MB/core.
  - Per core, per cam: view the [16, 196000] shard as [125p, 16b, 1568]
    where partition p holds classes 8p..8p+7 (1568 = 8 classes * 196 hw,
    contiguous in DRAM -> 6.3 KB DMA runs). One batch row per DMA load;
    DVE reduce_sum each tile [125, 8, 196] -> [125, 8] per-class partials,
    then a tiny second reduce over the 16 batch rows -> per-class sums.
  - Loads are striped across TWO DMA queues -- the SP (sync) HWDGE ring
    (f32) and the gpsimd SWDGE ring, which CASTS its rows to bf16 in the
    DMA so DVE reduces them in 16-bit 2x mode. Together the rings
    sustain ~850 GB/s/core (one ring alone: ~400); the bf16 tiles halve
    DVE cost, which is otherwise the critical path (f32 full-touch
    ~94 us/iter). The stripe gives the bf16 ring 9/16 of rows to balance
    DVE against both rings. bf16 quantization of ~56% of the inputs
    perturbs the loss by ~6e-5 relative -- far inside the 2e-2 gate. The
    Activation HWDGE ring is ~3.7x slower for bulk loads and carries
    only the tiny output stores, so no load ring ever stalls on DVE.
    (An act_cast=True variant where the Activation ALU re-casts the f32
    tiles to bf16 measured ~16 us/iter WORSE -- the serialized cast
    chain adds more critical-path latency than it removes from DVE.)
  - Stage1 uses a 3-D access pattern ([p, cc, xx], contiguous stage
    writes); the 4-D batched form costs ~2x on hardware.
  - One [125, 8] f32 output DMA per cam per core; host sums the 8 core
    partials, adds bincount(target), and computes the 4 scalar losses.

Raw Bass Block (not Tile): HWDGE DMA instructions only support one inline
sync-wait, so semaphores are placed by hand -- one completion sem per SBUF
slot (concurrent DMAs always target distinct slots), WAR on slot reuse
guarded transitively through the DVE sem.
"""

import numpy as np

B, C, H, W = 128, 1000, 14, 14
HWSZ = H * W                 # 196
N_CORES = 8
B_SH = B // N_CORES          # 16 batch rows per core
P = 125                      # partitions used; class c -> (p=c//8, cc=c%8)
CC = 8                       # classes per partition
RUN = CC * HWSZ              # 1568 contiguous f32 per (p, b)
F = C * HWSZ                 # 196000 elements per batch row
N_CAMS = 4

N_BUFS = 16                  # SBUF slots (6272 B/partition each)
LOAD_RINGS = ("sync", "gpsimd")
# ring assignment per (load index % 16): 0 -> sync (f32), 1 -> gpsimd
# (casts to bf16). 7:9 split — the bf16 ring takes slightly more than half
# because its tiles cost DVE half as much, balancing DVE against both DMA
# rings (measured: sync ~400 GB/s, gpsimd ~430 GB/s, DVE ~1 f32 or
# 2 bf16 elems/lane/cycle).
STRIPE = (0, 1, 0, 1, 1, 0, 1, 0, 1, 0, 1, 1, 0, 1, 0, 1)
# act_cast mode uses an even split instead: with the Act engine re-casting
# the sync ring's f32 tiles to bf16 (one Copy op per tile into a rotating
# scratch pool), DVE reduces every tile in 16-bit 2x mode and stops being
# the critical path; the rings rebalance to 8:8.
STRIPE_EVEN = (0, 1) * 8
N_SCRATCH = 8                # rotating bf16 scratch tiles for act_cast

_CACHE = {}


def _build_nc(n_bufs=None, n_iters=1, load_rings=None, cast_ring=True,
              act_cast=False):
    from contextlib import ExitStack

    import concourse.bass as bass
    import concourse.mybir as mybir

    nb = N_BUFS if n_bufs is None else n_bufs
    rings = LOAD_RINGS if load_rings is None else load_rings
    # cast_ring: the gpsimd (SWDGE) ring casts its rows to bf16 in the DMA;
    # DVE reduces those tiles in 16-bit mode (2x) into f32 stages. Ring
    # assignment follows STRIPE (period 16); nb must be 16 so that a slot
    # is always refilled by the same ring (same dtype).
    assert not cast_ring or (
        len(rings) == 2 and rings[1] == "gpsimd" and nb == 16
    )
    n_chunks = B_SH                  # loads per cam (one batch row each)
    n_loads = N_CAMS * n_chunks      # loads per iteration
    dve_per_iter = n_loads + N_CAMS  # stage1 + stage2 ops per iteration

    def dve_after_s1(k):
        # dve_sem value right after stage1-reduce #k retires (DVE order per
        # cam: n_chunks * s1 then one s2)
        return k + k // n_chunks + 1

    def dve_after_s2(i):
        # dve_sem delta within an iteration once cam i's stage2 retired
        return (i + 1) * (n_chunks + 1)

    f32 = mybir.dt.float32
    bf16 = mybir.dt.bfloat16
    nc = bass.Bass()
    cams = [
        nc.dram_tensor(f"cam_{i}", [B_SH, F], f32, kind="ExternalInput")
        for i in range(N_CAMS)
    ]
    out = nc.dram_tensor("sums", [P, N_CAMS * CC], f32, kind="ExternalOutput")

    stripe = STRIPE_EVEN if act_cast else STRIPE

    def ring_of(n):
        if cast_ring:
            return stripe[n % 16]
        return n % len(rings)

    def slot_dtype(s):
        # slot s is always filled by ring stripe[s % 16] (n_loads % 16 == 0
        # and nb == 16, so load n -> slot n % 16 preserves the stripe)
        return bf16 if cast_ring and stripe[s % 16] == 1 else f32

    def is_f32_tile(c):
        return cast_ring and stripe[c % 16] == 0

    def act_pos(c):
        # rank of f32 tile c within its cam's f32 tiles (act-cast order)
        return sum(1 for c2 in range(c) if stripe[c2 % 16] == 0)

    n_f32 = sum(1 for c in range(16) if stripe[c] == 0)  # f32 tiles per cam

    with ExitStack() as ctx:
        bufs = [
            ctx.enter_context(nc.sbuf_tensor(f"t{s}", [P, RUN], slot_dtype(s)))
            for s in range(nb)
        ]
        stages = [
            ctx.enter_context(nc.sbuf_tensor(f"stage{i}", [P, n_chunks, CC], f32))
            for i in range(N_CAMS)
        ]
        out_sums = ctx.enter_context(nc.sbuf_tensor("osum", [P, N_CAMS * CC], f32))
        scratch = [
            ctx.enter_context(nc.sbuf_tensor(f"sc{s}", [P, RUN], bf16))
            for s in range(N_SCRATCH if act_cast else 0)
        ]
        # one completion sem per buffer slot: concurrent loads target distinct
        # slots, so "slot_sem >= 16*k" unambiguously means "k-th load into this
        # slot is fully complete" (each DMA is 16 sub-completions)
        slot_sems = [
            ctx.enter_context(nc.semaphore(f"slot_sem{s}")) for s in range(nb)
        ]
        out_sem = ctx.enter_context(nc.semaphore("out_sem"))
        dve_sem = ctx.enter_context(nc.semaphore("dve_sem"))
        act_sem = ctx.enter_context(nc.semaphore("act_sem"))
        block = ctx.enter_context(nc.Block())

        act_per_iter = N_CAMS * n_f32  # Act cast ops per iteration

        def act_count_after(g, i, c):
            # act_sem value once the cast of tile (iter g, cam i, chunk c)
            # has retired (Act processes f32 tiles in load order)
            return g * act_per_iter + i * n_f32 + act_pos(c) + 1

        def loader(eng, g, parity, nrings):
            # emit this engine's share of iteration g's loads (striped);
            # slot-reuse WAR is guarded transitively through whichever
            # engine consumes the slot (DVE reduce, or Act cast when
            # act_cast handles this f32 slot)
            for n in range(n_loads):
                if ring_of(n) != parity:
                    continue
                i, c = divmod(n, n_chunks)
                gn = g * n_loads + n
                if gn >= nb:
                    pk = gn - nb
                    pg, pn = divmod(pk, n_loads)
                    pi, pc = divmod(pn, n_chunks)
                    if act_cast and is_f32_tile(pc):
                        eng.wait_ge(act_sem, act_count_after(pg, pi, pc))
                    else:
                        eng.wait_ge(
                            dve_sem, pg * dve_per_iter + dve_after_s1(pn)
                        )
                src = cams[i][c:c + 1, :].rearrange(
                    "b (p x) -> p (b x)", p=P, x=RUN
                )
                eng.dma_start(bufs[gn % nb][:], src).then_inc(
                    slot_sems[gn % nb], 16
                )

        for r, ring in enumerate(rings):

            def ring_body(eng, r=r):
                for g in range(n_iters):
                    loader(eng, g, r, len(rings))

            getattr(block, ring)(ring_body)

        @block.scalar
        def _(scalar):
            for g in range(n_iters):
                if act_cast:
                    # re-cast each f32 tile to bf16 scratch so DVE reduces
                    # it in 16-bit 2x mode; one Copy op per tile
                    t = 0
                    for i in range(N_CAMS):
                        for c in range(n_chunks):
                            if not is_f32_tile(c):
                                continue
                            gt = g * act_per_iter + t
                            if gt >= N_SCRATCH:
                                # WAR: scratch slot's previous tile was
                                # stage1-reduced by DVE
                                pt = gt - N_SCRATCH
                                pg, pr = divmod(pt, act_per_iter)
                                pi, pp = divmod(pr, n_f32)
                                pc = [c2 for c2 in range(n_chunks)
                                      if is_f32_tile(c2)][pp]
                                scalar.wait_ge(
                                    dve_sem,
                                    pg * dve_per_iter
                                    + dve_after_s1(pi * n_chunks + pc),
                                )
                            gn = g * n_loads + i * n_chunks + c
                            scalar.wait_ge(
                                slot_sems[gn % nb], 16 * (gn // nb + 1)
                            )
                            nc.scalar.copy(
                                out=scratch[gt % N_SCRATCH][:],
                                in_=bufs[gn % nb][:],
                            ).then_inc(act_sem, 1)
                            t += 1
                # per-cam output stores, last iteration only (out_sums is
                # rewritten every iteration; only the final value is read,
                # and per-iter stores would barrier this queue between
                # iterations in act_cast mode)
                if g == n_iters - 1:
                    for i in range(N_CAMS):
                        scalar.wait_ge(
                            dve_sem, g * dve_per_iter + dve_after_s2(i)
                        )
                        scalar.dma_start(
                            out[:, i * CC:(i + 1) * CC],
                            out_sums[:, i * CC:(i + 1) * CC],
                        ).then_inc(out_sem, 16)
            scalar.wait_ge(out_sem, 16 * N_CAMS)

        @block.vector
        def _(vector):
            for g in range(n_iters):
                dve_base = g * dve_per_iter
                for i in range(N_CAMS):
                    for c in range(n_chunks):
                        n = i * n_chunks + c
                        gn = g * n_loads + n
                        if g > 0 and c == 0:
                            # WAR: stages[i] reread by prev iter's stage2
                            vector.wait_ge(
                                dve_sem,
                                (g - 1) * dve_per_iter + dve_after_s2(i),
                            )
                        if act_cast and is_f32_tile(c):
                            # tile was re-cast to bf16 scratch by Act
                            gt = g * act_per_iter + i * n_f32 + act_pos(c)
                            vector.wait_ge(
                                act_sem, act_count_after(g, i, c)
                            )
                            src_tile = scratch[gt % N_SCRATCH]
                        else:
                            vector.wait_ge(
                                slot_sems[gn % nb], 16 * (gn // nb + 1)
                            )
                            src_tile = bufs[gn % nb]
                        nc.vector.reduce_sum(
                            out=stages[i][:, c],
                            in_=src_tile[:].rearrange(
                                "p (cc xx) -> p cc xx", cc=CC
                            ),
                            axis=mybir.AxisListType.X,
                        ).then_inc(dve_sem, 1)
                    # reduce the 16 batch partials per class:
                    # [P, cc, h] -> [P, cc]; same-engine wait makes sure the
                    # stage1 writes retired before this read
                    vector.wait_ge(dve_sem, dve_base + (i + 1) * n_chunks + i)
                    nc.vector.reduce_sum(
                        out=out_sums[:, i * CC:(i + 1) * CC],
                        in_=stages[i][:].rearrange("p h cc -> p cc h"),
                        axis=mybir.AxisListType.X,
                    ).then_inc(dve_sem, 1)

    return nc


def _get_nc():
    if "nc" not in _CACHE:
        _CACHE["nc"] = _build_nc()
    return _CACHE["nc"]


def _run_on_device(in_maps, nc=None, **kwargs):
    from concourse.bass_utils import run_bass_kernel_spmd

    return run_bass_kernel_spmd(
        nc if nc is not None else _get_nc(),
        in_maps,
        core_ids=list(range(N_CORES)),
        **kwargs,
    )


def _make_in_maps(cams):
    in_maps = []
    for k in range(N_CORES):
        m = {}
        for i, cam in enumerate(cams):
            m[f"cam_{i}"] = np.ascontiguousarray(
                np.asarray(cam)[k * B_SH:(k + 1) * B_SH].reshape(B_SH, F),
                dtype=np.float32,
            )
        in_maps.append(m)
    return in_maps


def kernel(cam_0, cam_1, cam_2, cam_3, target, _bench_results=None, **_kw):
    in_maps = _make_in_maps((cam_0, cam_1, cam_2, cam_3))
    res = _run_on_device(in_maps)
    if _bench_results is not None:
        _bench_results.append(res)

    # host combine: [125, 32] per core -> per-class totals -> scalar losses
    counts = np.bincount(np.asarray(target).astype(np.int64), minlength=C)
    avg_count = counts.astype(np.float64) / B
    total = np.zeros((P, N_CAMS * CC), dtype=np.float64)
    for r in res.results:
        total += r["sums"].astype(np.float64)

    losses = []
    for i in range(N_CAMS):
        per_class = total[:, i * CC:(i + 1) * CC].reshape(C)  # index = 8p+cc = c
        avg_conf = per_class / (B * HWSZ)
        losses.append(np.float32(np.abs(avg_conf - avg_count).mean()))
    return tuple(np.asarray(l, dtype=np.float32) for l in losses)
